# revision 30
# baseline (speedup 1.0000x reference)
"""Trainium2 Bass kernel for 16-head self-attention (N=4, S=2048, E=1024).

Sharding: 8 cores = 4 batches x 2 head-groups (8 heads each).  Each core
computes its head-group's attention and a partial fc_out product; the host
sums the two partials per batch, transposes, and adds the bias.

v2 design (vs baseline): weight folding moves both the Q and V projections
off-device; every matmul uses an M=128 bf16 stationary operand (FWL-eligible).

  scores = Xq @ (Xk @ Wk.T @ Wq).T / 32      (Q-proj folded into K side)
  ctx'   = softmax(scores) @ Xv               (raw values)
  out    = sum_h ctx'_h @ (Wo_h @ Wv).T + bo  (V-proj folded into Wo)

Device layout (per core):
  - qT/xkT: (4, 128, S) bf16 -- head-pair dims on partitions
  - wqk: (128, 128) bf16 blockdiag(Wk.T Wq x2); K-proj = 16 matmuls
  - vp tiles: (128, 8*128) bf16 per key-block: per head [ones(64) | v(64)]
    so ctx matmul (M=128, FWL) yields sumexp in psum rows 0-63 (base 0,
    required by the custom-DVE reciprocal) and ctx dims in rows 64-127
  - exp on ScalarE (scale=1/32 fused), (128,1024) bf16 out
  - normalization: reciprocal_approx_fast on sumexp rows + tensor_mul
    directly from PSUM -> bf16 ctxT (no DMA round trips)
  - fc_out: woT2 (128,128) bf16 stationary blocks, ctxT moving, transposed
    (embed-major) bf16 output; host transposes back
"""

import numpy as np
import ml_dtypes

_LDW_OPT = False  # walrus rejects explicit InstLdweights under ldw-opt


def _enable_ldw_opt():
    """Flip walrus's --enable-ldw-opt (weight-load optimization) on for our
    compiles; hidden weight loads are ~40% of the ctx matmul cost here."""
    try:
        from concourse import bass_utils
    except ImportError:
        return
    if getattr(bass_utils, "_ldw_opt_patched", False):
        return
    orig = bass_utils.run_command

    def patched(cmd, *a, **kw):
        if isinstance(cmd, list):
            cmd = [
                "--enable-ldw-opt=true" if c == "--enable-ldw-opt=false" else c
                for c in cmd
            ]
        return orig(cmd, *a, **kw)

    bass_utils.run_command = patched
    bass_utils._ldw_opt_patched = True


EMBED = 1024
HEADS = 16
HD = 64  # head dim
N_CORES = 8
HPC = 8  # heads per core
GCOLS = HPC * HD  # 512 embed columns per core


def ts(i, n):
    return slice(i * n, (i + 1) * n)


def build_program(S):
    import concourse.bass as bass
    import concourse.tile as tile
    import concourse.mybir as mybir
    from concourse import bacc

    f32 = mybir.dt.float32
    bf16 = mybir.dt.bfloat16
    i16 = mybir.dt.int16
    EXP = mybir.ActivationFunctionType.Exp

    NPAIR = 4  # head-pair blocks (2 heads each)
    NJB = S // 128  # key blocks
    NCC = S // 512  # query chunks
    NOC = EMBED // 128  # fc_out embed blocks

    # exp is column-split per tile: ScalarE handles [0:ACT_COLS] exactly,
    # the DVE handles [ACT_COLS:1024] concurrently via the Schraudolph
    # bit-trick: bf16(e^x) bits ~= int16(x*A + B)
    ACT_COLS = 1024
    SCH_A = 128.0 * 1.4426950408889634 / 32.0  # log2(e)*128, /32 score scale
    SCH_B = 128.0 * (127.0 - 0.0579)

    nc = bacc.Bacc("TRN2", target_bir_lowering=False, debug=False)

    qT_d = nc.dram_tensor("qT", [NPAIR, 128, S], bf16, kind="ExternalInput").ap()
    xkT_d = nc.dram_tensor("xkT", [NPAIR, 128, S], bf16, kind="ExternalInput").ap()
    wqk_d = nc.dram_tensor("wqk", [128, 128], bf16, kind="ExternalInput").ap()
    vpv_d = nc.dram_tensor("vpv", [NJB, 128, HPC * 128], bf16, kind="ExternalInput").ap()
    wo_d = nc.dram_tensor("woT2", [NPAIR, 128, EMBED], bf16, kind="ExternalInput").ap()
    out_d = nc.dram_tensor("outT", [EMBED, S], bf16, kind="ExternalOutput").ap()

    inv_sqrt_e = 1.0 / float(np.sqrt(EMBED))

    with tile.TileContext(nc) as tc:
        import contextlib

        with contextlib.ExitStack() as ctx:
            # ---- persistent pools ----
            const_p = ctx.enter_context(tc.tile_pool(name="const", bufs=1))
            qk_p = ctx.enter_context(tc.tile_pool(name="qk", bufs=1))
            vp_p = ctx.enter_context(tc.tile_pool(name="vp", bufs=1))
            ctxT_p = ctx.enter_context(tc.tile_pool(name="ctxT", bufs=1))
            # PSUM: sc slots 2 banks x2 bufs + ctx slots 1 bank x4 bufs = 8
            sc_ps = ctx.enter_context(tc.tile_pool(name="sc", bufs=2, space="PSUM"))
            ctx_ps = ctx.enter_context(tc.tile_pool(name="ctxps", bufs=4, space="PSUM"))
            exp_p = ctx.enter_context(tc.tile_pool(name="exp", bufs=8))
            rrs_p = ctx.enter_context(tc.tile_pool(name="rrs", bufs=4))
            fco_p = ctx.enter_context(tc.tile_pool(name="fco", bufs=4))

            # ---- weights / persistent tiles ----
            wqk_s = const_p.tile([128, 128], bf16, tag="wqk")
            wo_t = [const_p.tile([128, EMBED], bf16, tag=f"wo{p}", name=f"wo{p}") for p in range(NPAIR)]
            qT = [qk_p.tile([128, S], bf16, tag=f"qT{p}", name=f"qT{p}") for p in range(NPAIR)]
            kT = [qk_p.tile([128, S], bf16, tag=f"kT{p}", name=f"kT{p}") for p in range(NPAIR)]
            vp_t = [vp_p.tile([128, HPC * 128], bf16, tag=f"vp{jb}", name=f"vp{jb}") for jb in range(NJB)]
            ctxT = [ctxT_p.tile([128, S], bf16, tag=f"cx{p}", name=f"cx{p}") for p in range(NPAIR)]

            # ---- input DMAs: chunked + spread across trigger queues so the
            # first proj/scores/ctx work isn't gated on whole-tensor loads.
            # qT chunk triggers go first on the gpsimd queue; vp ones-memsets
            # follow the triggers there so nothing delays the critical path
            # (kT casts run on an empty Vector queue). ----
            prefetch = const_p.tile([1, 8], f32, tag="pref")
            nc.vector.memset(prefetch[:], 0.25)
            nc.scalar.activation(prefetch[0:1, 4:8], prefetch[0:1, 0:4], EXP)
            nc.gpsimd.dma_start(wqk_s[:], wqk_d[:])
            with tc.tile_pool(name="xin", bufs=1) as xin_p:
                xk = [xin_p.tile([128, S], bf16, tag=f"xk{p}", name=f"xk{p}") for p in range(NPAIR)]

                def vpv_dma(jb):
                    nc.gpsimd.dma_start(vp_t[jb][:], vpv_d[jb])

                # phase A: only what the first pair's proj + attention needs;
                # startup is DMA-bandwidth-bound, so everything else is gated
                # behind a dummy op that waits for the first kT cast
                for p in range(NPAIR):
                    for c in range(NCC):
                        eng = nc.sync if (p * NCC + c) % 2 == 0 else nc.scalar
                        eng.dma_start(xk[p][:, ts(c, 512)], xkT_d[p][:, ts(c, 512)])
                nc.gpsimd.dma_start(qT[0][:], qT_d[0])
                gate = const_p.tile([1, 8], bf16, tag="gate")
                nc.gpsimd.tensor_copy(gate[:], kT[0][0:1, 0:8])
                for jb in range(NJB):
                    vpv_dma(jb)
                for p in range(1, NPAIR):
                    nc.gpsimd.dma_start(qT[p][:], qT_d[p])
                for p in range(NPAIR):
                    nc.gpsimd.dma_start(wo_t[p][:], wo_d[p])

                # ---- K projection: kT = blockdiag(Wqk,Wqk).T @ xkT ----
                for pr in range(NPAIR):
                    for chp in range(S // 1024):
                        ps = sc_ps.tile([128, 1024], f32, tag="sc")
                        for c2 in range(2):
                            nc.tensor.matmul(
                                ps[:, ts(c2, 512)],
                                lhsT=wqk_s[:],
                                rhs=xk[pr][:, ts(chp * 2 + c2, 512)],
                                start=True,
                                stop=True,
                            )
                        nc.vector.tensor_copy(kT[pr][:, ts(chp, 1024)], ps[:])

            # ---- attention main loop ----
            # fc_out(cc) matmuls are interleaved into attention(cc+1)'s jb
            # loop (one oc-group per 8 iters) so the PE never blocks the
            # scores->exp stream for long; fc_out(last cc) runs as a tail.
            COPY = mybir.ActivationFunctionType.Copy

            def emit_fc(cc, oc):
                ps = ctx_ps.tile([128, 512], f32, tag="ctx")
                for p in range(NPAIR):
                    nc.tensor.matmul(
                        ps[:],
                        lhsT=wo_t[p][:, ts(oc, 128)],
                        rhs=ctxT[p][:, ts(cc, 512)],
                        start=(p == 0),
                        stop=(p == NPAIR - 1),
                    )
                fo = fco_p.tile([128, 512], bf16, tag="fco")
                if cc == NCC - 1 and oc % 2 == 1:
                    # ScalarE is idle after the last exp; share the tail copies
                    nc.scalar.activation(fo[:], ps[:], COPY)
                else:
                    nc.vector.tensor_copy(fo[:], ps[:])
                eng = nc.gpsimd if oc % 2 == 0 else nc.sync
                eng.dma_start(out_d[ts(oc, 128), ts(cc, 512)], fo[:])

            # normalization ops are spread one-per-iteration into the next
            # pair's jb loop so the Vector queue never bursts (a burst stalls
            # the DVE share of exp, which stalls the whole pipeline)
            def make_norm(pr, cc, hl, cps):
                def op_recip():
                    rrs = rrs_p.tile([64, 512], f32, tag="rrs")
                    nc.vector.reciprocal_approx_fast(rrs[:], cps[0:64, :])
                    return rrs

                def op_mult(rrs):
                    nc.vector.tensor_mul(
                        ctxT[pr][hl * 64 : hl * 64 + 64, ts(cc, 512)],
                        cps[64:128, :],
                        rrs[:],
                    )

                return op_recip, op_mult

            norm_queue = []

            def pop_norm():
                if norm_queue:
                    norm_queue.pop(0)()

            for cc in range(NCC):
                fc_queue = list(range(NOC)) if cc > 0 else []
                it = 0
                for pr in range(NPAIR):
                    cpsA = ctx_ps.tile([128, 512], f32, tag="ctx", name=f"cA{pr}_{cc}")
                    cpsB = ctx_ps.tile([128, 512], f32, tag="ctx", name=f"cB{pr}_{cc}")
                    for jb in range(NJB):
                        s_t = sc_ps.tile([128, 1024], f32, tag="sc")
                        for hl, b in ((0, 0), (1, 64)):
                            nc.tensor.matmul(
                                s_t[:, ts(hl, 512)],
                                lhsT=kT[pr][b : b + 64, ts(jb, 128)],
                                rhs=qT[pr][b : b + 64, ts(cc, 512)],
                                start=True,
                                stop=True,
                            )
                        e_t = exp_p.tile([128, 1024], bf16, tag="exp")
                        if ACT_COLS < 1024:
                            nc.scalar.activation(
                                e_t[:, 0:ACT_COLS],
                                s_t[:, 0:ACT_COLS],
                                EXP,
                                scale=inv_sqrt_e,
                            )
                            nc.vector.tensor_scalar(
                                e_t[:, ACT_COLS:1024].bitcast(i16),
                                s_t[:, ACT_COLS:1024],
                                SCH_A,
                                SCH_B,
                                mybir.AluOpType.mult,
                                mybir.AluOpType.add,
                            )
                        else:
                            nc.scalar.activation(e_t[:], s_t[:], EXP, scale=inv_sqrt_e)
                        e_ap = e_t[:]
                        for hl, cps in ((0, cpsA), (1, cpsB)):
                            hh = pr * 2 + hl
                            nc.tensor.matmul(
                                cps[:],
                                lhsT=vp_t[jb][:, ts(hh, 128)],
                                rhs=e_ap[:, ts(hl, 512)],
                                start=(jb == 0),
                                stop=(jb == NJB - 1),
                            )
                        it += 1
                        if fc_queue and it % 8 == 0:
                            emit_fc(cc - 1, fc_queue.pop(0))
                    # normalization: rows 0-63 = sumexp copies, 64-127 = ctx
                    for hl, cps in ((0, cpsA), (1, cpsB)):
                        op_recip, op_mult = make_norm(pr, cc, hl, cps)
                        op_mult(op_recip())
                assert not fc_queue
            for oc in range(NOC):
                emit_fc(NCC - 1, oc)

    nc.compile()
    return nc


def make_core_inputs(values, keys, queries, Wv, Wk, Wq, Wo, n, g, S):
    """Host-side marshaling for core (n, g): transpose/cast slices + weight folds."""
    bf = ml_dtypes.bfloat16
    cols = slice(g * GCOLS, (g + 1) * GCOLS)
    NPAIR = 4
    NJB = S // 128

    def xt(x):
        t = np.ascontiguousarray(x[n][:, cols].T.astype(bf))  # (512, S)
        return t.reshape(NPAIR, 128, S)

    # K-side fold: scores = Xq @ (Xk @ Wk.T @ Wq).T
    wqk_small = (Wk.T @ Wq).astype(np.float32)  # (64, 64)
    wqk = np.zeros((128, 128), np.float32)
    wqk[0:64, 0:64] = wqk_small
    wqk[64:128, 64:128] = wqk_small

    # V-side fold: out partial = sum_h ctx'_h @ (Wo_h @ Wv).T
    woT2 = np.empty((NPAIR, 128, EMBED), np.float32)
    for p in range(NPAIR):
        for hl in range(2):
            h = g * HPC + 2 * p + hl
            wov = Wo[:, h * HD : (h + 1) * HD] @ Wv  # (1024, 64)
            woT2[p, hl * 64 : hl * 64 + 64, :] = wov.T

    v4 = values[n][:, cols].astype(bf).reshape(NJB, 128, HPC, HD)
    vpv = np.ones((NJB, 128, HPC, 2 * HD), bf)
    vpv[:, :, :, HD:] = v4
    vpv = np.ascontiguousarray(vpv).reshape(NJB, 128, HPC * 2 * HD)

    return {
        "qT": xt(queries),
        "xkT": xt(keys),
        "wqk": wqk.astype(bf),
        "vpv": vpv,
        "woT2": woT2.astype(bf),
    }


_PROG_CACHE = {}
TRACE = False
LAST_RESULTS = None


def kernel(values, keys, queries, mask, Wv, Wk, Wq, Wo, bo):
    global LAST_RESULTS
    from concourse.bass_utils import run_bass_kernel_spmd

    values = np.asarray(values, np.float32)
    keys = np.asarray(keys, np.float32)
    queries = np.asarray(queries, np.float32)
    Wv = np.asarray(Wv, np.float32)
    Wk = np.asarray(Wk, np.float32)
    Wq = np.asarray(Wq, np.float32)
    Wo = np.asarray(Wo, np.float32)
    bo = np.asarray(bo, np.float32)

    N, S, _ = queries.shape
    if S not in _PROG_CACHE:
        if _LDW_OPT:
            _enable_ldw_opt()
        _PROG_CACHE[S] = build_program(S)
    nc = _PROG_CACHE[S]

    in_maps = [
        make_core_inputs(values, keys, queries, Wv, Wk, Wq, Wo, c // 2, c % 2, S)
        for c in range(N_CORES)
    ]
    res = run_bass_kernel_spmd(
        nc, in_maps, core_ids=list(range(N_CORES)), trace=TRACE
    )
    LAST_RESULTS = res
    out = np.empty((N, S, EMBED), np.float32)
    for n in range(N):
        acc = res.results[2 * n]["outT"].astype(np.float32)
        acc += res.results[2 * n + 1]["outT"].astype(np.float32)
        out[n] = acc.T + bo
    return out


# revision 31
# speedup vs baseline: 1.0167x; 1.0167x over previous
"""Trainium2 Bass kernel for 16-head self-attention (N=4, S=2048, E=1024).

Sharding: 8 cores = 4 batches x 2 head-groups (8 heads each).  Each core
computes its head-group's attention and a partial fc_out product; the host
sums the two partials per batch, transposes, and adds the bias.

v2 design (vs baseline): weight folding moves both the Q and V projections
off-device; every matmul uses an M=128 bf16 stationary operand (FWL-eligible).

  scores = Xq @ (Xk @ Wk.T @ Wq).T / 32      (Q-proj folded into K side)
  ctx'   = softmax(scores) @ Xv               (raw values)
  out    = sum_h ctx'_h @ (Wo_h @ Wv).T + bo  (V-proj folded into Wo)

Device layout (per core):
  - qT/xkT: (4, 128, S) bf16 -- head-pair dims on partitions
  - wqk: (128, 128) bf16 blockdiag(Wk.T Wq x2); K-proj = 16 matmuls
  - vp tiles: (128, 8*128) bf16 per key-block: per head [ones(64) | v(64)]
    so ctx matmul (M=128, FWL) yields sumexp in psum rows 0-63 (base 0,
    required by the custom-DVE reciprocal) and ctx dims in rows 64-127
  - exp on ScalarE (scale=1/32 fused), (128,1024) bf16 out
  - normalization: reciprocal_approx_fast on sumexp rows + tensor_mul
    directly from PSUM -> bf16 ctxT (no DMA round trips)
  - fc_out: woT2 (128,128) bf16 stationary blocks, ctxT moving, transposed
    (embed-major) bf16 output; host transposes back
"""

import numpy as np
import ml_dtypes

_LDW_OPT = False  # walrus rejects explicit InstLdweights under ldw-opt


def _enable_ldw_opt():
    """Flip walrus's --enable-ldw-opt (weight-load optimization) on for our
    compiles; hidden weight loads are ~40% of the ctx matmul cost here."""
    try:
        from concourse import bass_utils
    except ImportError:
        return
    if getattr(bass_utils, "_ldw_opt_patched", False):
        return
    orig = bass_utils.run_command

    def patched(cmd, *a, **kw):
        if isinstance(cmd, list):
            cmd = [
                "--enable-ldw-opt=true" if c == "--enable-ldw-opt=false" else c
                for c in cmd
            ]
        return orig(cmd, *a, **kw)

    bass_utils.run_command = patched
    bass_utils._ldw_opt_patched = True


EMBED = 1024
HEADS = 16
HD = 64  # head dim
N_CORES = 8
HPC = 8  # heads per core
GCOLS = HPC * HD  # 512 embed columns per core


def ts(i, n):
    return slice(i * n, (i + 1) * n)


def build_program(S):
    import concourse.bass as bass
    import concourse.tile as tile
    import concourse.mybir as mybir
    from concourse import bacc

    f32 = mybir.dt.float32
    bf16 = mybir.dt.bfloat16
    i16 = mybir.dt.int16
    EXP = mybir.ActivationFunctionType.Exp

    NPAIR = 4  # head-pair blocks (2 heads each)
    NJB = S // 128  # key blocks
    NCC = S // 512  # query chunks
    NOC = EMBED // 128  # fc_out embed blocks

    # exp is column-split per tile: ScalarE handles [0:ACT_COLS] exactly,
    # the DVE handles [ACT_COLS:1024] concurrently via the Schraudolph
    # bit-trick: bf16(e^x) bits ~= int16(x*A + B)
    ACT_COLS = 1024
    SCH_A = 128.0 * 1.4426950408889634 / 32.0  # log2(e)*128, /32 score scale
    SCH_B = 128.0 * (127.0 - 0.0579)

    nc = bacc.Bacc("TRN2", target_bir_lowering=False, debug=False)

    qT_d = nc.dram_tensor("qT", [NPAIR, 128, S], bf16, kind="ExternalInput").ap()
    xkT_d = nc.dram_tensor("xkT", [NPAIR, 128, S], bf16, kind="ExternalInput").ap()
    wqk_d = nc.dram_tensor("wqk", [128, 128], bf16, kind="ExternalInput").ap()
    vpv_d = nc.dram_tensor("vpv", [NJB, 128, GCOLS], bf16, kind="ExternalInput").ap()
    wo_d = nc.dram_tensor("woT2", [NPAIR, 128, EMBED], bf16, kind="ExternalInput").ap()
    out_d = nc.dram_tensor("outT", [EMBED, S], bf16, kind="ExternalOutput").ap()

    inv_sqrt_e = 1.0 / float(np.sqrt(EMBED))

    with tile.TileContext(nc) as tc:
        import contextlib

        with contextlib.ExitStack() as ctx:
            # ---- persistent pools ----
            const_p = ctx.enter_context(tc.tile_pool(name="const", bufs=1))
            qk_p = ctx.enter_context(tc.tile_pool(name="qk", bufs=1))
            vp_p = ctx.enter_context(tc.tile_pool(name="vp", bufs=1))
            ctxT_p = ctx.enter_context(tc.tile_pool(name="ctxT", bufs=1))
            # PSUM: sc slots 2 banks x2 bufs + ctx slots 1 bank x4 bufs = 8
            sc_ps = ctx.enter_context(tc.tile_pool(name="sc", bufs=2, space="PSUM"))
            ctx_ps = ctx.enter_context(tc.tile_pool(name="ctxps", bufs=4, space="PSUM"))
            exp_p = ctx.enter_context(tc.tile_pool(name="exp", bufs=8))
            rrs_p = ctx.enter_context(tc.tile_pool(name="rrs", bufs=4))
            fco_p = ctx.enter_context(tc.tile_pool(name="fco", bufs=4))

            # ---- weights / persistent tiles ----
            wqk_s = const_p.tile([128, 128], bf16, tag="wqk")
            wo_t = [const_p.tile([128, EMBED], bf16, tag=f"wo{p}", name=f"wo{p}") for p in range(NPAIR)]
            qT = [qk_p.tile([128, S], bf16, tag=f"qT{p}", name=f"qT{p}") for p in range(NPAIR)]
            kT = [qk_p.tile([128, S], bf16, tag=f"kT{p}", name=f"kT{p}") for p in range(NPAIR)]
            vp_t = [vp_p.tile([128, HPC * 128], bf16, tag=f"vp{jb}", name=f"vp{jb}") for jb in range(NJB)]
            ctxT = [ctxT_p.tile([128, S], bf16, tag=f"cx{p}", name=f"cx{p}") for p in range(NPAIR)]

            # ---- input DMAs: chunked + spread across trigger queues so the
            # first proj/scores/ctx work isn't gated on whole-tensor loads.
            # qT chunk triggers go first on the gpsimd queue; vp ones-memsets
            # follow the triggers there so nothing delays the critical path
            # (kT casts run on an empty Vector queue). ----
            prefetch = const_p.tile([1, 8], f32, tag="pref")
            nc.vector.memset(prefetch[:], 0.25)
            nc.scalar.activation(prefetch[0:1, 4:8], prefetch[0:1, 0:4], EXP)
            # vp ones halves on the idle early Vector queue
            for jb in range(NJB):
                vpr = vp_t[jb].rearrange("p (h c) -> p h c", c=128)
                nc.vector.memset(vpr[:, :, 0:64], 1.0)
            nc.sync.dma_start(wqk_s[:], wqk_d[:])
            with tc.tile_pool(name="xin", bufs=1) as xin_p:
                xk = [xin_p.tile([128, S], bf16, tag=f"xk{p}", name=f"xk{p}") for p in range(NPAIR)]

                def vpv_dma(jb):
                    vpr = vp_t[jb].rearrange("p (h c) -> p h c", c=128)
                    nc.gpsimd.dma_start(
                        vpr[:, :, 64:128],
                        vpv_d[jb].rearrange("p (h c) -> p h c", c=64),
                    )

                # phase A: only what the first pair's proj + attention needs;
                # startup is DMA-bandwidth-bound, so everything else is gated
                # behind a dummy op that waits for the first kT cast
                for p in range(NPAIR):
                    nc.sync.dma_start(xk[p][:], xkT_d[p])
                nc.gpsimd.dma_start(qT[0][:], qT_d[0])
                for jb in range(3):
                    vpv_dma(jb)
                gate = const_p.tile([1, 8], bf16, tag="gate")
                nc.gpsimd.tensor_copy(gate[:], kT[0][0:1, 0:8])
                for jb in range(3, NJB):
                    vpv_dma(jb)
                for p in range(1, NPAIR):
                    nc.gpsimd.dma_start(qT[p][:], qT_d[p])
                for p in range(NPAIR):
                    nc.gpsimd.dma_start(wo_t[p][:], wo_d[p])

                # ---- K projection: kT = blockdiag(Wqk,Wqk).T @ xkT ----
                for pr in range(NPAIR):
                    for chp in range(S // 1024):
                        ps = sc_ps.tile([128, 1024], f32, tag="sc")
                        for c2 in range(2):
                            nc.tensor.matmul(
                                ps[:, ts(c2, 512)],
                                lhsT=wqk_s[:],
                                rhs=xk[pr][:, ts(chp * 2 + c2, 512)],
                                start=True,
                                stop=True,
                            )
                        nc.vector.tensor_copy(kT[pr][:, ts(chp, 1024)], ps[:])

            # ---- attention main loop ----
            # fc_out(cc) matmuls are interleaved into attention(cc+1)'s jb
            # loop (one oc-group per 8 iters) so the PE never blocks the
            # scores->exp stream for long; fc_out(last cc) runs as a tail.
            COPY = mybir.ActivationFunctionType.Copy

            def emit_fc(cc, oc):
                ps = ctx_ps.tile([128, 512], f32, tag="ctx")
                for p in range(NPAIR):
                    nc.tensor.matmul(
                        ps[:],
                        lhsT=wo_t[p][:, ts(oc, 128)],
                        rhs=ctxT[p][:, ts(cc, 512)],
                        start=(p == 0),
                        stop=(p == NPAIR - 1),
                    )
                fo = fco_p.tile([128, 512], bf16, tag="fco")
                if cc == NCC - 1 and oc % 2 == 1:
                    # ScalarE is idle after the last exp; share the tail copies
                    nc.scalar.activation(fo[:], ps[:], COPY)
                else:
                    nc.vector.tensor_copy(fo[:], ps[:])
                eng = nc.gpsimd if oc % 2 == 0 else nc.sync
                eng.dma_start(out_d[ts(oc, 128), ts(cc, 512)], fo[:])

            # normalization ops are spread one-per-iteration into the next
            # pair's jb loop so the Vector queue never bursts (a burst stalls
            # the DVE share of exp, which stalls the whole pipeline)
            def make_norm(pr, cc, hl, cps):
                def op_recip():
                    rrs = rrs_p.tile([64, 512], f32, tag="rrs")
                    nc.vector.reciprocal_approx_fast(rrs[:], cps[0:64, :])
                    return rrs

                def op_mult(rrs):
                    nc.vector.tensor_mul(
                        ctxT[pr][hl * 64 : hl * 64 + 64, ts(cc, 512)],
                        cps[64:128, :],
                        rrs[:],
                    )

                return op_recip, op_mult

            norm_queue = []

            def pop_norm():
                if norm_queue:
                    norm_queue.pop(0)()

            for cc in range(NCC):
                fc_queue = list(range(NOC)) if cc > 0 else []
                it = 0
                for pr in range(NPAIR):
                    cpsA = ctx_ps.tile([128, 512], f32, tag="ctx", name=f"cA{pr}_{cc}")
                    cpsB = ctx_ps.tile([128, 512], f32, tag="ctx", name=f"cB{pr}_{cc}")
                    for jb in range(NJB):
                        s_t = sc_ps.tile([128, 1024], f32, tag="sc")
                        for hl, b in ((0, 0), (1, 64)):
                            nc.tensor.matmul(
                                s_t[:, ts(hl, 512)],
                                lhsT=kT[pr][b : b + 64, ts(jb, 128)],
                                rhs=qT[pr][b : b + 64, ts(cc, 512)],
                                start=True,
                                stop=True,
                            )
                        e_t = exp_p.tile([128, 1024], bf16, tag="exp")
                        if ACT_COLS < 1024:
                            nc.scalar.activation(
                                e_t[:, 0:ACT_COLS],
                                s_t[:, 0:ACT_COLS],
                                EXP,
                                scale=inv_sqrt_e,
                            )
                            nc.vector.tensor_scalar(
                                e_t[:, ACT_COLS:1024].bitcast(i16),
                                s_t[:, ACT_COLS:1024],
                                SCH_A,
                                SCH_B,
                                mybir.AluOpType.mult,
                                mybir.AluOpType.add,
                            )
                        else:
                            nc.scalar.activation(e_t[:], s_t[:], EXP, scale=inv_sqrt_e)
                        e_ap = e_t[:]
                        for hl, cps in ((0, cpsA), (1, cpsB)):
                            hh = pr * 2 + hl
                            nc.tensor.matmul(
                                cps[:],
                                lhsT=vp_t[jb][:, ts(hh, 128)],
                                rhs=e_ap[:, ts(hl, 512)],
                                start=(jb == 0),
                                stop=(jb == NJB - 1),
                            )
                        it += 1
                        if fc_queue and it % 8 == 0:
                            emit_fc(cc - 1, fc_queue.pop(0))
                    # normalization: rows 0-63 = sumexp copies, 64-127 = ctx
                    for hl, cps in ((0, cpsA), (1, cpsB)):
                        op_recip, op_mult = make_norm(pr, cc, hl, cps)
                        op_mult(op_recip())
                assert not fc_queue
            for oc in range(NOC):
                emit_fc(NCC - 1, oc)

    nc.compile()
    return nc


def make_core_inputs(values, keys, queries, Wv, Wk, Wq, Wo, n, g, S):
    """Host-side marshaling for core (n, g): transpose/cast slices + weight folds."""
    bf = ml_dtypes.bfloat16
    cols = slice(g * GCOLS, (g + 1) * GCOLS)
    NPAIR = 4
    NJB = S // 128

    def xt(x):
        t = np.ascontiguousarray(x[n][:, cols].T.astype(bf))  # (512, S)
        return t.reshape(NPAIR, 128, S)

    # K-side fold: scores = Xq @ (Xk @ Wk.T @ Wq).T
    wqk_small = (Wk.T @ Wq).astype(np.float32)  # (64, 64)
    wqk = np.zeros((128, 128), np.float32)
    wqk[0:64, 0:64] = wqk_small
    wqk[64:128, 64:128] = wqk_small

    # V-side fold: out partial = sum_h ctx'_h @ (Wo_h @ Wv).T
    woT2 = np.empty((NPAIR, 128, EMBED), np.float32)
    for p in range(NPAIR):
        for hl in range(2):
            h = g * HPC + 2 * p + hl
            wov = Wo[:, h * HD : (h + 1) * HD] @ Wv  # (1024, 64)
            woT2[p, hl * 64 : hl * 64 + 64, :] = wov.T

    vpv = np.ascontiguousarray(values[n][:, cols].astype(bf)).reshape(NJB, 128, GCOLS)

    return {
        "qT": xt(queries),
        "xkT": xt(keys),
        "wqk": wqk.astype(bf),
        "vpv": vpv,
        "woT2": woT2.astype(bf),
    }


_PROG_CACHE = {}
TRACE = False
LAST_RESULTS = None


def kernel(values, keys, queries, mask, Wv, Wk, Wq, Wo, bo):
    global LAST_RESULTS
    from concourse.bass_utils import run_bass_kernel_spmd

    values = np.asarray(values, np.float32)
    keys = np.asarray(keys, np.float32)
    queries = np.asarray(queries, np.float32)
    Wv = np.asarray(Wv, np.float32)
    Wk = np.asarray(Wk, np.float32)
    Wq = np.asarray(Wq, np.float32)
    Wo = np.asarray(Wo, np.float32)
    bo = np.asarray(bo, np.float32)

    N, S, _ = queries.shape
    if S not in _PROG_CACHE:
        if _LDW_OPT:
            _enable_ldw_opt()
        _PROG_CACHE[S] = build_program(S)
    nc = _PROG_CACHE[S]

    in_maps = [
        make_core_inputs(values, keys, queries, Wv, Wk, Wq, Wo, c // 2, c % 2, S)
        for c in range(N_CORES)
    ]
    res = run_bass_kernel_spmd(
        nc, in_maps, core_ids=list(range(N_CORES)), trace=TRACE
    )
    LAST_RESULTS = res
    out = np.empty((N, S, EMBED), np.float32)
    for n in range(N):
        acc = res.results[2 * n]["outT"].astype(np.float32)
        acc += res.results[2 * n + 1]["outT"].astype(np.float32)
        out[n] = acc.T + bo
    return out
